# revision 16
# baseline (speedup 1.0000x reference)
"""CAM (channel self-attention) kernel for Trainium2 — 8 NeuronCores, batch-parallel.

Math per batch element b (A = x[b] reshaped [N=4096, C=512]):
    G = A^T A                  [C, C]   (symmetric)
    P = softmax_rows(G)        [C, C]
    Y = A P                    [N, C]
    out = gamma * Y + x

Sharding: data-parallel over batch — core i handles batch element i.

Design notes:
  - fp8e4 (E4M3) DoubleRow matmuls: contract TWO 128-row k-tiles per
    pass (2x bf16 MAC throughput). f32 PSUM accumulation; f32 epilogue
    keeps the residual path exact.
  - Partition-contiguous layout "(p t) c": partition p holds rows
    32p..32p+31, so DMA groups move large contiguous runs/partition.
    Input/output groups alternate between the two HWDGE rings
    (sync/scalar) to overlap per-group issue bubbles; both schedules
    taper at the edges (small first groups so compute starts early,
    small last groups to shrink the serial tails).
  - Gram: upper-triangle only (free dims 512/384/256/128 per row-block,
    saving ~40% of Gram PE time); the lower triangle is reconstructed
    after the last accumulation with 6 f32 PE transposes while rows are
    assembled into SBUF for softmax.
  - A^T for the Y phase: fp8 PE transposes staged in PSUM, one u16 copy
    per chunk pair (stride-2 pad matches fp8 transpose output step).
    Transposes of the last DEFER chunks are emitted after the load loop
    so they fill the otherwise-idle PE window during softmax instead of
    delaying the last Gram matmul.
  - Softmax per row-block: DVE row-max (negated) -> ACT exp with fused
    row-sum -> DVE reciprocal -> ACT copy-scale to fp8 P.
  - Y: 2 DoubleRow matmuls/chunk, y banks rotate through a 6-buffer
    PSUM pool (recycling warm+Gram banks); epilogue gamma*Y + x on DVE.
"""

import numpy as np

import concourse.tile as tile
from concourse import bacc, mybir
from concourse.bass_utils import run_bass_kernel_spmd
from concourse.masks import make_identity

B = 8
H = 64
W = 64
C = 512
HW = H * W            # 4096 rows per batch element
NT = HW // 128        # 32 row chunks of 128 (chunk k = rows {32p + k})
CT = C // 128         # 4
DEFER = 6             # trailing chunks whose transposes run during softmax

F32 = mybir.dt.float32
U16 = mybir.dt.uint16
FP8 = mybir.dt.float8e4
DR = mybir.MatmulPerfMode.DoubleRow

_CACHE = {}


def _emit(nc, tc, out, x, gamma):
    from contextlib import ExitStack

    with ExitStack() as ctx:
        big = ctx.enter_context(tc.tile_pool(name="big", bufs=1))
        small = ctx.enter_context(tc.tile_pool(name="small", bufs=1))
        stat = ctx.enter_context(tc.tile_pool(name="stat", bufs=4))
        sbstage = ctx.enter_context(tc.tile_pool(name="sbstage", bufs=6))
        ostage = ctx.enter_context(tc.tile_pool(name="ostage", bufs=4))
        ps = ctx.enter_context(tc.tile_pool(name="ps", bufs=6, space="PSUM"))
        tps = ctx.enter_context(tc.tile_pool(name="tps", bufs=2, space="PSUM"))

        A32 = big.tile([128, NT, C], F32)       # x rows, row 32p+t on part p
        A8 = big.tile([128, NT, C], FP8)        # fp8 cast of A32
        # A^T, stride-2 padded (fp8 PE transposes write with element step 2;
        # the pad lets the PSUM->SBUF copy run as contiguous u16):
        # AT8[p, ci, k, j, 0] = A[32j+k, 128ci+p]
        AT8 = big.tile([128, CT, NT, 128, 2], FP8)
        G32 = big.tile([128, CT, C], F32)       # assembled full Gram rows
        E32 = big.tile([128, CT, C], F32)       # exp(G - rowmax)
        P8 = big.tile([128, CT, C], FP8)        # softmax(G) in fp8

        ident8 = small.tile([128, 128], FP8)
        make_identity(nc, ident8[:])
        ident32 = small.tile([128, 128], F32)
        make_identity(nc, ident32[:])

        gB = small.tile([128, 1], F32)          # gamma broadcast to partitions

        # PE warm-up: HAM clock gate holds the PE slow until it has been
        # busy a while; burn the DMA lead-in with short dummy matmuls.
        warm8 = small.tile([128, 2, C], FP8)
        nc.gpsimd.memset(warm8[:], 0.0)
        warm_ps = ps.tile([128, C], F32, name="ps", tag="ps")
        NW = 8
        for wi in range(NW):
            nc.tensor.matmul(
                warm_ps[:, 0:256], warm8[:, :, 0:128], warm8[:, :, 0:256],
                start=(wi == 0), stop=(wi == NW - 1), perf_mode=DR,
            )

        # Upper-triangle Gram accumulators: g0 (512) and g2 (256) own a
        # bank; g1 (384) and g3 (128) share the third (disjoint regions).
        gb0 = ps.tile([128, C], F32, name="ps", tag="ps")
        gb13 = ps.tile([128, C], F32, name="ps", tag="ps")
        gb2 = ps.tile([128, C], F32, name="ps", tag="ps")
        g_up = [gb0[:], gb13[:, 0:384], gb2[:, 0:256], gb13[:, 384:512]]

        def emit_transposes(k2):
            # 4 fp8 transposes of chunk k2 into a half of the pair tile
            tp = _tp[0]
            if k2 % 2 == 0:
                tp = tps.tile([128, 2, CT, 128, 2], FP8, name="tp", tag="tp")
                _tp[0] = tp
            j2 = k2 % 2
            for ci in range(CT):
                nc.tensor.transpose(
                    tp[:, j2, ci, :, 0],
                    A8[:, k2, ci * 128:(ci + 1) * 128],
                    ident8[:],
                )
            if j2 == 1:
                kk = k2 - 1
                nc.scalar.copy(
                    AT8[:, :, kk:kk + 2, :, :]
                    .rearrange("p ci k j two -> p k ci j two")
                    .bitcast(U16),
                    tp[:].bitcast(U16),
                )
        _tp = [None]

        xr = x.rearrange("(p t) c -> p t c", t=NT)

        load_groups = [2, 2, 4, 8, 8, 4, 2, 1, 1]
        assert sum(load_groups) == NT
        k0 = 0
        for gi, gsz in enumerate(load_groups):
            ieng = nc.sync if gi % 2 == 0 else nc.scalar
            ieng.dma_start(A32[:, k0:k0 + gsz, :], xr[:, k0:k0 + gsz, :])
            if gi == 0:
                nc.scalar.dma_start(gB[:], gamma[:])
            for j in range(gsz):
                k = k0 + j
                # cast f32 -> fp8 (DVE)
                nc.vector.tensor_copy(A8[:, k, :], A32[:, k, :])
                if k % 2 == 1:
                    kk = k - 1
                    # Gram first: upper-triangle DoubleRow matmuls; the
                    # last of these gates softmax.
                    for mi in range(CT):
                        nc.tensor.matmul(
                            g_up[mi],
                            A8[:, kk:kk + 2, mi * 128:(mi + 1) * 128],
                            A8[:, kk:kk + 2, mi * 128:],
                            start=(kk == 0), stop=(kk == NT - 2),
                            perf_mode=DR,
                            skip_group_check=(mi % 2 == 1),
                        )
                    if kk < NT - DEFER:
                        emit_transposes(kk)
                        emit_transposes(kk + 1)
            k0 += gsz

        # Assemble full Gram rows in SBUF. Step 1: drain every read of the
        # g banks first (off-diag blocks to SBUF staging + upper-row
        # copies), so the later lb/y allocations may recycle those banks.
        OFFD = [(mi, j) for mi in range(1, CT) for j in range(mi)]
        sbs = []
        for n, (mi, j) in enumerate(OFFD):
            sb = sbstage.tile([128, 128], F32)
            nc.scalar.copy(
                sb[:], g_up[j][:, (mi - j) * 128:(mi - j + 1) * 128])
            sbs.append(sb)
        for mi in range(CT):
            if mi % 2 == 0:
                nc.vector.tensor_copy(G32[:, mi, mi * 128:], g_up[mi])
            else:
                nc.scalar.copy(G32[:, mi, mi * 128:], g_up[mi])
        # Step 2: f32 PE transposes of the staged blocks into recycled
        # PSUM banks, copied out into the lower-triangle slots.
        for n, (mi, j) in enumerate(OFFD):
            lb = ps.tile([128, C], F32, name="ps", tag="ps")
            nc.tensor.transpose(lb[:, 0:128], sbs[n][:], ident32[:])
            if (mi + j) % 2 == 0:
                nc.vector.tensor_copy(
                    G32[:, mi, j * 128:(j + 1) * 128], lb[:, 0:128])
            else:
                nc.scalar.copy(
                    G32[:, mi, j * 128:(j + 1) * 128], lb[:, 0:128])

        # Deferred transposes land in the PE queue between the recon
        # transposes and the first Y matmul — they execute during softmax.
        for k2 in range(NT - DEFER, NT):
            emit_transposes(k2)

        # softmax over rows of G (free axis)
        for mi in range(CT):
            nmax = stat.tile([128, 1], F32)
            nc.vector.tensor_reduce(
                nmax[:], G32[:, mi, :],
                axis=mybir.AxisListType.X, op=mybir.AluOpType.max, negate=True,
            )
            esum = stat.tile([128, 1], F32)
            nc.scalar.activation(
                E32[:, mi, :], G32[:, mi, :],
                mybir.ActivationFunctionType.Exp,
                bias=nmax[:], scale=1.0, accum_out=esum[:],
            )
            rsum = stat.tile([128, 1], F32)
            nc.vector.reciprocal(rsum[:], esum[:])
            nc.scalar.activation(
                P8[:, mi, :], E32[:, mi, :],
                mybir.ActivationFunctionType.Copy, scale=rsum[:],
            )

        # Y = A @ P (DoubleRow, 2 matmuls/chunk), epilogue gamma*Y + x.
        out_r = out.rearrange("(p t) c -> p t c", t=NT)
        out_groups = [1, 1, 2, 4, 4, 4, 4, 4, 4, 2, 1, 1]
        assert sum(out_groups) == NT
        t0 = 0
        for h, osz in enumerate(out_groups):
            o32 = ostage.tile([128, 4, C], F32)
            for j in range(osz):
                t = t0 + j
                y = ps.tile([128, C], F32, name="ps", tag="ps")
                for cp in range(CT // 2):
                    nc.tensor.matmul(
                        y[:],
                        AT8[:, 2 * cp:2 * cp + 2, t, :, 0],
                        P8[:, 2 * cp:2 * cp + 2, :],
                        start=(cp == 0), stop=(cp == CT // 2 - 1),
                        perf_mode=DR,
                    )
                nc.vector.scalar_tensor_tensor(
                    o32[:, j, :], y[:], gB[:], A32[:, t, :],
                    op0=mybir.AluOpType.mult, op1=mybir.AluOpType.add,
                )
            oeng = nc.sync if h % 2 == 0 else nc.scalar
            oeng.dma_start(out_r[:, t0:t0 + osz, :], o32[:, 0:osz, :])
            t0 += osz


def build():
    nc = bacc.Bacc("TRN2", target_bir_lowering=False, debug=False)
    x = nc.dram_tensor("x", [HW, C], F32, kind="ExternalInput").ap()
    gamma = nc.dram_tensor("gamma", [128, 1], F32, kind="ExternalInput").ap()
    out = nc.dram_tensor("out", [HW, C], F32, kind="ExternalOutput").ap()
    with tile.TileContext(nc) as tc:
        _emit(nc, tc, out, x, gamma)
    nc.compile()
    return nc


def kernel(x: np.ndarray, gamma: np.ndarray, trace: bool = False):
    assert x.shape == (B, H, W, C), x.shape
    if "nc" not in _CACHE:
        _CACHE["nc"] = build()
    nc = _CACHE["nc"]

    g128 = np.full((128, 1), np.float32(np.asarray(gamma).reshape(-1)[0]),
                   dtype=np.float32)
    in_maps = [
        {
            "x": np.ascontiguousarray(
                np.asarray(x[i], dtype=np.float32).reshape(HW, C)),
            "gamma": g128,
        }
        for i in range(B)
    ]
    if trace:
        res = run_bass_kernel_spmd(nc, in_maps, core_ids=list(range(B)),
                                   trace=True)
    else:
        # Force-untraced: a stray BASS_TRACE in the environment would route
        # through profiling hooks this image may not have.
        import os
        prev = os.environ.get("BASS_NEVER_TRACE")
        os.environ["BASS_NEVER_TRACE"] = "1"
        try:
            res = run_bass_kernel_spmd(nc, in_maps, core_ids=list(range(B)))
        finally:
            if prev is None:
                os.environ.pop("BASS_NEVER_TRACE", None)
            else:
                os.environ["BASS_NEVER_TRACE"] = prev
    _CACHE["last_result"] = res
    out = np.stack([res.results[i]["out"] for i in range(B)], axis=0)
    return out.reshape(B, H, W, C).astype(np.float32)


# revision 19
# speedup vs baseline: 1.1940x; 1.1940x over previous
"""CAM (channel self-attention) kernel for Trainium2 — 8 NeuronCores, batch-parallel.

Math per batch element b (A = x[b] reshaped [N=4096, C=512]):
    G = A^T A                  [C, C]   (symmetric)
    P = softmax_rows(G)        [C, C]
    Y = A P                    [N, C]
    out = gamma * Y + x

Sharding: data-parallel over batch — core i handles batch element i.

Design notes:
  - fp8e4 (E4M3) DoubleRow matmuls: contract TWO 128-row k-tiles per
    pass (2x bf16 MAC throughput). f32 PSUM accumulation; f32 epilogue
    keeps the residual path exact.
  - Partition-contiguous layout "(p t) c": partition p holds rows
    32p..32p+31, so DMA groups move large contiguous runs/partition.
    Input/output groups alternate between the two HWDGE rings
    (sync/scalar) to overlap per-group issue bubbles; both schedules
    taper at the edges (small first groups so compute starts early,
    small last groups to shrink the serial tails).
  - Gram: upper-triangle only (free dims 512/384/256/128 per row-block,
    saving ~40% of Gram PE time); the lower triangle is reconstructed
    after the last accumulation with 6 f32 PE transposes while rows are
    assembled into SBUF for softmax.
  - A^T for the Y phase: fp8 PE transposes staged in PSUM, one u16 copy
    per chunk pair (stride-2 pad matches fp8 transpose output step).
    Transposes of the last DEFER chunks are emitted after the load loop
    so they fill the otherwise-idle PE window during softmax instead of
    delaying the last Gram matmul.
  - Softmax per row-block: DVE row-max (negated) -> ACT exp with fused
    row-sum -> DVE reciprocal -> ACT copy-scale to fp8 P.
  - Y: 2 DoubleRow matmuls/chunk, y banks rotate through a 6-buffer
    PSUM pool (recycling warm+Gram banks); epilogue gamma*Y + x on DVE.
"""

import numpy as np

import concourse.tile as tile
from concourse import bacc, mybir
from concourse.bass_utils import run_bass_kernel_spmd
from concourse.masks import make_identity

B = 8
H = 64
W = 64
C = 512
HW = H * W            # 4096 rows per batch element
NT = HW // 128        # 32 row chunks of 128 (chunk k = rows {32p + k})
CT = C // 128         # 4
DEFER = 6             # trailing chunks whose transposes run during softmax

F32 = mybir.dt.float32
U16 = mybir.dt.uint16
FP8 = mybir.dt.float8e4
DR = mybir.MatmulPerfMode.DoubleRow

_CACHE = {}


def _emit(nc, tc, out, x, gamma):
    from contextlib import ExitStack

    with ExitStack() as ctx:
        big = ctx.enter_context(tc.tile_pool(name="big", bufs=1))
        small = ctx.enter_context(tc.tile_pool(name="small", bufs=1))
        stat = ctx.enter_context(tc.tile_pool(name="stat", bufs=4))
        sbstage = ctx.enter_context(tc.tile_pool(name="sbstage", bufs=6))
        ostage = ctx.enter_context(tc.tile_pool(name="ostage", bufs=4))
        ps = ctx.enter_context(tc.tile_pool(name="ps", bufs=6, space="PSUM"))
        tps = ctx.enter_context(tc.tile_pool(name="tps", bufs=2, space="PSUM"))

        A32 = big.tile([128, NT, C], F32)       # x rows, row 32p+t on part p
        A8 = big.tile([128, NT, C], FP8)        # fp8 cast of A32
        # A^T, stride-2 padded (fp8 PE transposes write with element step 2;
        # the pad lets the PSUM->SBUF copy run as contiguous u16):
        # AT8[p, ci, k, j, 0] = A[32j+k, 128ci+p]
        AT8 = big.tile([128, CT, NT, 128, 2], FP8)
        G32 = big.tile([128, CT, C], F32)       # assembled full Gram rows
        E32 = big.tile([128, CT, C], F32)       # exp(G - rowmax)
        P8 = big.tile([128, CT, C], FP8)        # softmax(G) in fp8

        ident8 = small.tile([128, 128], FP8)
        make_identity(nc, ident8[:])
        ident32 = small.tile([128, 128], F32)
        make_identity(nc, ident32[:])

        gB = small.tile([128, 1], F32)          # gamma broadcast to partitions

        # PE warm-up: HAM clock gate holds the PE slow until it has been
        # busy a while; burn the DMA lead-in with short dummy matmuls.
        warm8 = small.tile([128, 2, C], FP8)
        nc.gpsimd.memset(warm8[:], 0.0)
        warm_ps = ps.tile([128, C], F32, name="ps", tag="ps")
        NW = 8
        for wi in range(NW):
            nc.tensor.matmul(
                warm_ps[:, 0:256], warm8[:, :, 0:128], warm8[:, :, 0:256],
                start=(wi == 0), stop=(wi == NW - 1), perf_mode=DR,
            )

        # Upper-triangle Gram accumulators: g0 (512) and g2 (256) own a
        # bank; g1 (384) and g3 (128) share the third (disjoint regions).
        gb0 = ps.tile([128, C], F32, name="ps", tag="ps")
        gb13 = ps.tile([128, C], F32, name="ps", tag="ps")
        gb2 = ps.tile([128, C], F32, name="ps", tag="ps")
        g_up = [gb0[:], gb13[:, 0:384], gb2[:, 0:256], gb13[:, 384:512]]

        def emit_transposes(k2):
            # 4 fp8 transposes of chunk k2 into a half of the pair tile
            tp = _tp[0]
            if k2 % 2 == 0:
                tp = tps.tile([128, 2, CT, 128, 2], FP8, name="tp", tag="tp")
                _tp[0] = tp
            j2 = k2 % 2
            for ci in range(CT):
                nc.tensor.transpose(
                    tp[:, j2, ci, :, 0],
                    A8[:, k2, ci * 128:(ci + 1) * 128],
                    ident8[:],
                )
            if j2 == 1:
                kk = k2 - 1
                nc.scalar.copy(
                    AT8[:, :, kk:kk + 2, :, :]
                    .rearrange("p ci k j two -> p k ci j two")
                    .bitcast(U16),
                    tp[:].bitcast(U16),
                )
        _tp = [None]

        xr = x.rearrange("(p t) c -> p t c", t=NT)

        load_groups = [2, 2, 4, 8, 8, 4, 2, 1, 1]
        assert sum(load_groups) == NT
        k0 = 0
        for gi, gsz in enumerate(load_groups):
            # input stays on the sync ring: the ACT ring's in-order queue
            # would stall DMA issues behind the AT copies
            nc.sync.dma_start(A32[:, k0:k0 + gsz, :], xr[:, k0:k0 + gsz, :])
            if gi == 0:
                nc.scalar.dma_start(gB[:], gamma[:])
            for j in range(gsz):
                k = k0 + j
                # cast f32 -> fp8 (DVE)
                nc.vector.tensor_copy(A8[:, k, :], A32[:, k, :])
                if k % 2 == 1:
                    kk = k - 1
                    # Gram first: upper-triangle DoubleRow matmuls; the
                    # last of these gates softmax.
                    for mi in range(CT):
                        nc.tensor.matmul(
                            g_up[mi],
                            A8[:, kk:kk + 2, mi * 128:(mi + 1) * 128],
                            A8[:, kk:kk + 2, mi * 128:],
                            start=(kk == 0), stop=(kk == NT - 2),
                            perf_mode=DR,
                            skip_group_check=(mi % 2 == 1),
                        )
                    if kk < NT - DEFER:
                        emit_transposes(kk)
                        emit_transposes(kk + 1)
            k0 += gsz

        # Assemble full Gram rows in SBUF. Step 1: drain every read of the
        # g banks first (off-diag blocks to SBUF staging + upper-row
        # copies), so the later lb/y allocations may recycle those banks.
        OFFD = [(mi, j) for mi in range(1, CT) for j in range(mi)]
        sbs = []
        for n, (mi, j) in enumerate(OFFD):
            sb = sbstage.tile([128, 128], F32)
            nc.scalar.copy(
                sb[:], g_up[j][:, (mi - j) * 128:(mi - j + 1) * 128])
            sbs.append(sb)
        for mi in range(CT):
            if mi % 2 == 0:
                nc.vector.tensor_copy(G32[:, mi, mi * 128:], g_up[mi])
            else:
                nc.scalar.copy(G32[:, mi, mi * 128:], g_up[mi])
        # Step 2: f32 PE transposes of the staged blocks into recycled
        # PSUM banks, copied out into the lower-triangle slots.
        for n, (mi, j) in enumerate(OFFD):
            lb = ps.tile([128, C], F32, name="ps", tag="ps")
            nc.tensor.transpose(lb[:, 0:128], sbs[n][:], ident32[:])
            if (mi + j) % 2 == 0:
                nc.vector.tensor_copy(
                    G32[:, mi, j * 128:(j + 1) * 128], lb[:, 0:128])
            else:
                nc.scalar.copy(
                    G32[:, mi, j * 128:(j + 1) * 128], lb[:, 0:128])

        # Deferred transposes land in the PE queue between the recon
        # transposes and the first Y matmul — they execute during softmax.
        for k2 in range(NT - DEFER, NT):
            emit_transposes(k2)

        # softmax over rows of G (free axis)
        for mi in range(CT):
            nmax = stat.tile([128, 1], F32)
            nc.vector.tensor_reduce(
                nmax[:], G32[:, mi, :],
                axis=mybir.AxisListType.X, op=mybir.AluOpType.max, negate=True,
            )
            esum = stat.tile([128, 1], F32)
            nc.scalar.activation(
                E32[:, mi, :], G32[:, mi, :],
                mybir.ActivationFunctionType.Exp,
                bias=nmax[:], scale=1.0, accum_out=esum[:],
            )
            rsum = stat.tile([128, 1], F32)
            nc.vector.reciprocal(rsum[:], esum[:])
            nc.vector.tensor_scalar_mul(P8[:, mi, :], E32[:, mi, :], rsum[:])

        # Y = A @ P (DoubleRow, 2 matmuls/chunk), epilogue gamma*Y + x.
        out_r = out.rearrange("(p t) c -> p t c", t=NT)
        out_groups = [1, 1, 2, 4, 4, 4, 4, 4, 4, 2, 1, 1]
        assert sum(out_groups) == NT
        t0 = 0
        for h, osz in enumerate(out_groups):
            o32 = ostage.tile([128, 4, C], F32)
            for j in range(osz):
                t = t0 + j
                y = ps.tile([128, C], F32, name="ps", tag="ps")
                for cp in range(CT // 2):
                    nc.tensor.matmul(
                        y[:],
                        AT8[:, 2 * cp:2 * cp + 2, t, :, 0],
                        P8[:, 2 * cp:2 * cp + 2, :],
                        start=(cp == 0), stop=(cp == CT // 2 - 1),
                        perf_mode=DR,
                    )
                nc.vector.scalar_tensor_tensor(
                    o32[:, j, :], y[:], gB[:], A32[:, t, :],
                    op0=mybir.AluOpType.mult, op1=mybir.AluOpType.add,
                )
            # last groups ride the idle ACT ring to dodge Sync-ring backlog
            oeng = nc.scalar if h >= len(out_groups) - 2 else nc.sync
            oeng.dma_start(out_r[:, t0:t0 + osz, :], o32[:, 0:osz, :])
            t0 += osz


def build():
    nc = bacc.Bacc("TRN2", target_bir_lowering=False, debug=False)
    x = nc.dram_tensor("x", [HW, C], F32, kind="ExternalInput").ap()
    gamma = nc.dram_tensor("gamma", [128, 1], F32, kind="ExternalInput").ap()
    out = nc.dram_tensor("out", [HW, C], F32, kind="ExternalOutput").ap()
    with tile.TileContext(nc) as tc:
        _emit(nc, tc, out, x, gamma)
    nc.compile()
    return nc


def kernel(x: np.ndarray, gamma: np.ndarray, trace: bool = False):
    assert x.shape == (B, H, W, C), x.shape
    if "nc" not in _CACHE:
        _CACHE["nc"] = build()
    nc = _CACHE["nc"]

    g128 = np.full((128, 1), np.float32(np.asarray(gamma).reshape(-1)[0]),
                   dtype=np.float32)
    in_maps = [
        {
            "x": np.ascontiguousarray(
                np.asarray(x[i], dtype=np.float32).reshape(HW, C)),
            "gamma": g128,
        }
        for i in range(B)
    ]
    if trace:
        res = run_bass_kernel_spmd(nc, in_maps, core_ids=list(range(B)),
                                   trace=True)
    else:
        # Force-untraced: a stray BASS_TRACE in the environment would route
        # through profiling hooks this image may not have.
        import os
        prev = os.environ.get("BASS_NEVER_TRACE")
        os.environ["BASS_NEVER_TRACE"] = "1"
        try:
            res = run_bass_kernel_spmd(nc, in_maps, core_ids=list(range(B)))
        finally:
            if prev is None:
                os.environ.pop("BASS_NEVER_TRACE", None)
            else:
                os.environ["BASS_NEVER_TRACE"] = prev
    _CACHE["last_result"] = res
    out = np.stack([res.results[i]["out"] for i in range(B)], axis=0)
    return out.reshape(B, H, W, C).astype(np.float32)


# revision 22
# speedup vs baseline: 1.3178x; 1.1037x over previous
"""CAM (channel self-attention) kernel for Trainium2 — 8 NeuronCores, batch-parallel.

Math per batch element b (A = x[b] reshaped [N=4096, C=512]):
    G = A^T A                  [C, C]   (symmetric)
    P = softmax_rows(G)        [C, C]
    Y = A P                    [N, C]
    out = gamma * Y + x

Sharding: data-parallel over batch — core i handles batch element i.

Design notes:
  - fp8e4 (E4M3) DoubleRow matmuls: contract TWO 128-row k-tiles per
    pass (2x bf16 MAC throughput). f32 PSUM accumulation; f32 epilogue
    keeps the residual path exact.
  - Partition-contiguous layout "(p t) c": partition p holds rows
    32p..32p+31, so DMA groups move large contiguous runs/partition.
    Input/output groups alternate between the two HWDGE rings
    (sync/scalar) to overlap per-group issue bubbles; both schedules
    taper at the edges (small first groups so compute starts early,
    small last groups to shrink the serial tails).
  - Gram: upper-triangle only (free dims 512/384/256/128 per row-block,
    saving ~40% of Gram PE time); the lower triangle is reconstructed
    after the last accumulation with 6 f32 PE transposes while rows are
    assembled into SBUF for softmax.
  - A^T for the Y phase: fp8 PE transposes staged in PSUM, one u16 copy
    per chunk pair (stride-2 pad matches fp8 transpose output step).
    Transposes of the last DEFER chunks are emitted after the load loop
    so they fill the otherwise-idle PE window during softmax instead of
    delaying the last Gram matmul.
  - Softmax per row-block: DVE row-max (negated) -> ACT exp with fused
    row-sum -> DVE reciprocal -> ACT copy-scale to fp8 P.
  - Y: 2 DoubleRow matmuls/chunk, y banks rotate through a 6-buffer
    PSUM pool (recycling warm+Gram banks); epilogue gamma*Y + x on DVE.
"""

import numpy as np

import concourse.tile as tile
from concourse import bacc, mybir
from concourse.bass_utils import run_bass_kernel_spmd
from concourse.masks import make_identity

B = 8
H = 64
W = 64
C = 512
HW = H * W            # 4096 rows per batch element
NT = HW // 128        # 32 row chunks of 128 (chunk k = rows {32p + k})
CT = C // 128         # 4
DEFER = 6             # trailing chunks whose transposes run during softmax

F32 = mybir.dt.float32
U16 = mybir.dt.uint16
FP8 = mybir.dt.float8e4
DR = mybir.MatmulPerfMode.DoubleRow

_CACHE = {}


def _emit(nc, tc, out, x, gamma):
    from contextlib import ExitStack

    with ExitStack() as ctx:
        big = ctx.enter_context(tc.tile_pool(name="big", bufs=1))
        small = ctx.enter_context(tc.tile_pool(name="small", bufs=1))
        stat = ctx.enter_context(tc.tile_pool(name="stat", bufs=4))
        sbstage = ctx.enter_context(tc.tile_pool(name="sbstage", bufs=6))
        ostage = ctx.enter_context(tc.tile_pool(name="ostage", bufs=6))
        ps = ctx.enter_context(tc.tile_pool(name="ps", bufs=6, space="PSUM"))
        tps = ctx.enter_context(tc.tile_pool(name="tps", bufs=2, space="PSUM"))

        A32 = big.tile([128, NT, C], F32)       # x rows, row 32p+t on part p
        A8 = big.tile([128, NT, C], FP8)        # fp8 cast of A32
        # A^T, stride-2 padded (fp8 PE transposes write with element step 2;
        # the pad lets the PSUM->SBUF copy run as contiguous u16):
        # AT8[p, ci, k, j, 0] = A[32j+k, 128ci+p]
        AT8 = big.tile([128, CT, NT, 128, 2], FP8)
        G32 = big.tile([128, CT, C], F32)       # assembled full Gram rows
        E32 = big.tile([128, CT, C], F32)       # exp(G - rowmax)
        P8 = big.tile([128, CT, C], FP8)        # softmax(G) in fp8

        ident8 = small.tile([128, 128], FP8)
        make_identity(nc, ident8[:])
        ident32 = small.tile([128, 128], F32)
        make_identity(nc, ident32[:])

        gB = small.tile([128, 1], F32)          # gamma broadcast to partitions

        # PE warm-up: HAM clock gate holds the PE slow until it has been
        # busy a while; burn the DMA lead-in with short dummy matmuls.
        warm8 = small.tile([128, 2, C], FP8)
        nc.gpsimd.memset(warm8[:], 0.0)
        warm_ps = ps.tile([128, C], F32, name="ps", tag="ps")
        NW = 8
        for wi in range(NW):
            nc.tensor.matmul(
                warm_ps[:, 0:256], warm8[:, :, 0:128], warm8[:, :, 0:256],
                start=(wi == 0), stop=(wi == NW - 1), perf_mode=DR,
            )

        # Upper-triangle Gram accumulators: g0 (512) and g2 (256) own a
        # bank; g1 (384) and g3 (128) share the third (disjoint regions).
        gb0 = ps.tile([128, C], F32, name="ps", tag="ps")
        gb13 = ps.tile([128, C], F32, name="ps", tag="ps")
        gb2 = ps.tile([128, C], F32, name="ps", tag="ps")
        g_up = [gb0[:], gb13[:, 0:384], gb2[:, 0:256], gb13[:, 384:512]]

        def emit_transposes(k2):
            # 4 fp8 transposes of chunk k2 into a half of the pair tile
            tp = _tp[0]
            if k2 % 2 == 0:
                tp = tps.tile([128, 2, CT, 128, 2], FP8, name="tp", tag="tp")
                _tp[0] = tp
            j2 = k2 % 2
            for ci in range(CT):
                nc.tensor.transpose(
                    tp[:, j2, ci, :, 0],
                    A8[:, k2, ci * 128:(ci + 1) * 128],
                    ident8[:],
                )
            if j2 == 1:
                kk = k2 - 1
                nc.scalar.copy(
                    AT8[:, :, kk:kk + 2, :, :]
                    .rearrange("p ci k j two -> p k ci j two")
                    .bitcast(U16),
                    tp[:].bitcast(U16),
                )
        _tp = [None]

        xr = x.rearrange("(p t) c -> p t c", t=NT)

        # Uniform small groups: the HWDGE streams queued batches back to
        # back, so fine granularity costs no bandwidth — but it makes the
        # per-group completion semaphores land smoothly, letting the
        # cast/Gram pipeline track the stream instead of piling into a
        # serial tail after a coarse late group completes.
        load_groups = [2] * 16
        assert sum(load_groups) == NT
        k0 = 0
        for gi, gsz in enumerate(load_groups):
            # input stays on the sync ring: the ACT ring's in-order queue
            # would stall DMA issues behind the AT copies
            nc.sync.dma_start(A32[:, k0:k0 + gsz, :], xr[:, k0:k0 + gsz, :])
            if gi == 0:
                nc.scalar.dma_start(gB[:], gamma[:])
            for j in range(gsz):
                k = k0 + j
                # cast f32 -> fp8 (DVE)
                nc.vector.tensor_copy(A8[:, k, :], A32[:, k, :])
                if k % 2 == 1:
                    kk = k - 1
                    # Gram first: upper-triangle DoubleRow matmuls; the
                    # last of these gates softmax.
                    for mi in range(CT):
                        nc.tensor.matmul(
                            g_up[mi],
                            A8[:, kk:kk + 2, mi * 128:(mi + 1) * 128],
                            A8[:, kk:kk + 2, mi * 128:],
                            start=(kk == 0), stop=(kk == NT - 2),
                            perf_mode=DR,
                            skip_group_check=(mi % 2 == 1),
                        )
                    if kk < NT - DEFER:
                        emit_transposes(kk)
                        emit_transposes(kk + 1)
            k0 += gsz

        # Assemble each full Gram row in SBUF and run its softmax as soon
        # as it completes. Row 0 is just its upper copy (no lower blocks),
        # so softmax starts ~immediately after the last Gram matmul; rows
        # 1-3 pipeline behind their f32 PE transposes of the upper blocks.
        # Emission-order invariant for PSUM recycling: every sb copy that
        # reads a g bank is emitted before the lb allocation that recycles
        # that bank (lb4..lb6 land on the g banks).
        for mi in range(CT):
            if mi % 2 == 0:
                nc.vector.tensor_copy(G32[:, mi, mi * 128:], g_up[mi])
            else:
                nc.scalar.copy(G32[:, mi, mi * 128:], g_up[mi])
            sbs = []
            for j in range(mi):
                sb = sbstage.tile([128, 128], F32)
                eng = nc.scalar if j % 2 == 0 else nc.vector
                if j % 2 == 0:
                    nc.scalar.copy(
                        sb[:], g_up[j][:, (mi - j) * 128:(mi - j + 1) * 128])
                else:
                    nc.vector.tensor_copy(
                        sb[:], g_up[j][:, (mi - j) * 128:(mi - j + 1) * 128])
                sbs.append(sb)
            for j in range(mi):
                lb = ps.tile([128, C], F32, name="ps", tag="ps")
                nc.tensor.transpose(lb[:, 0:128], sbs[j][:], ident32[:])
                if (mi + j) % 2 == 0:
                    nc.vector.tensor_copy(
                        G32[:, mi, j * 128:(j + 1) * 128], lb[:, 0:128])
                else:
                    nc.scalar.copy(
                        G32[:, mi, j * 128:(j + 1) * 128], lb[:, 0:128])
            # softmax of row mi
            nmax = stat.tile([128, 1], F32)
            nc.vector.tensor_reduce(
                nmax[:], G32[:, mi, :],
                axis=mybir.AxisListType.X, op=mybir.AluOpType.max, negate=True,
            )
            esum = stat.tile([128, 1], F32)
            nc.scalar.activation(
                E32[:, mi, :], G32[:, mi, :],
                mybir.ActivationFunctionType.Exp,
                bias=nmax[:], scale=1.0, accum_out=esum[:],
            )
            rsum = stat.tile([128, 1], F32)
            nc.vector.reciprocal(rsum[:], esum[:])
            nc.vector.tensor_scalar_mul(P8[:, mi, :], E32[:, mi, :], rsum[:])

        # Deferred A^T transposes: emitted after softmax so their PSUM->
        # SBUF copies queue on ACT behind the exps (not ahead of them);
        # the PE executes them during the softmax window, well before the
        # Y phase reaches chunk NT-DEFER.
        for k2 in range(NT - DEFER, NT):
            emit_transposes(k2)

        # Y = A @ P (DoubleRow, 2 matmuls/chunk), epilogue gamma*Y + x.
        out_r = out.rearrange("(p t) c -> p t c", t=NT)
        out_groups = [1, 1, 2, 4, 4, 4, 4, 4, 4, 2, 1, 1]
        assert sum(out_groups) == NT
        t0 = 0
        for h, osz in enumerate(out_groups):
            o32 = ostage.tile([128, 4, C], F32)
            for j in range(osz):
                t = t0 + j
                y = ps.tile([128, C], F32, name="ps", tag="ps")
                for cp in range(CT // 2):
                    nc.tensor.matmul(
                        y[:],
                        AT8[:, 2 * cp:2 * cp + 2, t, :, 0],
                        P8[:, 2 * cp:2 * cp + 2, :],
                        start=(cp == 0), stop=(cp == CT // 2 - 1),
                        perf_mode=DR,
                    )
                nc.vector.scalar_tensor_tensor(
                    o32[:, j, :], y[:], gB[:], A32[:, t, :],
                    op0=mybir.AluOpType.mult, op1=mybir.AluOpType.add,
                )
            # last groups ride the idle ACT ring to dodge Sync-ring backlog
            oeng = nc.scalar if h >= len(out_groups) - 2 else nc.sync
            oeng.dma_start(out_r[:, t0:t0 + osz, :], o32[:, 0:osz, :])
            t0 += osz


def build():
    nc = bacc.Bacc("TRN2", target_bir_lowering=False, debug=False)
    x = nc.dram_tensor("x", [HW, C], F32, kind="ExternalInput").ap()
    gamma = nc.dram_tensor("gamma", [128, 1], F32, kind="ExternalInput").ap()
    out = nc.dram_tensor("out", [HW, C], F32, kind="ExternalOutput").ap()
    with tile.TileContext(nc) as tc:
        _emit(nc, tc, out, x, gamma)
    nc.compile()
    return nc


def kernel(x: np.ndarray, gamma: np.ndarray, trace: bool = False):
    assert x.shape == (B, H, W, C), x.shape
    if "nc" not in _CACHE:
        _CACHE["nc"] = build()
    nc = _CACHE["nc"]

    g128 = np.full((128, 1), np.float32(np.asarray(gamma).reshape(-1)[0]),
                   dtype=np.float32)
    in_maps = [
        {
            "x": np.ascontiguousarray(
                np.asarray(x[i], dtype=np.float32).reshape(HW, C)),
            "gamma": g128,
        }
        for i in range(B)
    ]
    if trace:
        res = run_bass_kernel_spmd(nc, in_maps, core_ids=list(range(B)),
                                   trace=True)
    else:
        # Force-untraced: a stray BASS_TRACE in the environment would route
        # through profiling hooks this image may not have.
        import os
        prev = os.environ.get("BASS_NEVER_TRACE")
        os.environ["BASS_NEVER_TRACE"] = "1"
        try:
            res = run_bass_kernel_spmd(nc, in_maps, core_ids=list(range(B)))
        finally:
            if prev is None:
                os.environ.pop("BASS_NEVER_TRACE", None)
            else:
                os.environ["BASS_NEVER_TRACE"] = prev
    _CACHE["last_result"] = res
    out = np.stack([res.results[i]["out"] for i in range(B)], axis=0)
    return out.reshape(B, H, W, C).astype(np.float32)
